# revision 8
# baseline (speedup 1.0000x reference)
"""CTC loss kernel for Trainium2 (8 NeuronCores, batch-parallel).

Strategy
--------
Data-parallel over batch: each of the 8 cores handles 256 of the 2048
examples (2 "groups" of 128 partition-lanes).

Per core the CTC forward recursion is reformulated as 65 sequential
`tensor_tensor_scan` instructions over the extended-label axis s instead
of 255 tiny per-timestep updates:  for a fixed state s, the recursion
over time  a_t[s] = (a_{t-1}[s] + c_t[s]) * p_t[s]  with
c_t[s] = a_{t-1}[s-1] + m[s]*a_{t-1}[s-2] is exactly the DVE scan
primitive state=(data0+state)*data1 along the free dim, once the series
A_{s-1}, A_{s-2} (computed by the previous scans) are materialized.

Numerics: linear domain with exact softmax normalization folded into the
probabilities (p_hat = exp(logit) * e^kappa / Z_t), where kappa ~= the
mean per-step log-likelihood so alpha stays centered inside fp32 range
(validated: |log alpha| < 62 for this input distribution).
loss_b = T*kappa - log(sum of final alpha states).

Both batch groups ride in one scan instruction via a zero-probability
pad slot that resets the scan state between the groups.

Blank states (even s) all share one probability series (the blank row),
so only the 32 label series are gathered -- via `dma_gather` row-gathers
from a host-side transposed copy of the logits (rows are then
time-contiguous).
"""

import math
import os

import numpy as np

import concourse.bacc as bacc
import concourse.bass as bass
import concourse.mybir as mybir
from concourse import tile
from concourse.bass_utils import run_bass_kernel_spmd

# ---------------- problem constants (hardcoded per the task) -------------
B, T, C, L = 2048, 256, 55, 32
BLANK = C - 1            # 54
S = 2 * L + 1            # 65
NCORES = 8
BLOC = B // NCORES       # 256 examples per core
P = 128                  # partitions
G = 2                    # groups per core
W = 1 + T + 1 + T        # 514 slots: [init | g0 t=0..255 | pad | g1 t=0..255]
KAPPA = 3.73             # drift-centering constant (any value near the mean
                         # per-step log-lik works; exactness not required)
NJB = 8                  # label-series gather blocks
JPB = L // NJB           # 4 label series per block... (see below)

f32 = mybir.dt.float32
bf16 = mybir.dt.bfloat16
i16 = mybir.dt.int16

_CACHE = {}


def _build_nc():
    nc = bacc.Bacc("TRN2", target_bir_lowering=False, debug=False)
    dbg = bool(os.environ.get("CTC_DEBUG_DUMP"))
    nj = L // NJB                    # label series per gather block
    nidx = G * nj * P                # gather indices per block
    icols = nidx // 16               # idx tile columns per block
    lgt = nc.dram_tensor("lgt", [BLOC, C, T], f32, kind="ExternalInput")
    gidx = nc.dram_tensor("gidx", [P, NJB * icols], i16, kind="ExternalInput")
    mc = nc.dram_tensor("mc", [P, G, L], f32, kind="ExternalInput")
    fm = nc.dram_tensor("fm", [P, S, G], f32, kind="ExternalInput")
    lhat = nc.dram_tensor("lhat", [P, G], f32, kind="ExternalOutput")
    adump = (nc.dram_tensor("adump", [P, S, W], f32, kind="ExternalOutput")
             if dbg else None)

    AluOp = mybir.AluOpType
    Act = mybir.ActivationFunctionType

    with tile.TileContext(nc) as tc:
        with (
            tc.tile_pool(name="main", bufs=1) as pool,
            tc.tile_pool(name="gpool", bufs=2) as gpool,
        ):
            # ---------- load + exp(logits^T) ----------
            LT = pool.tile([P, G, C, T], bf16, tag="LT")
            src = lgt.ap().rearrange("(g p) c t -> p g c t", p=P)
            nc.gpsimd.dma_start(LT[:], src)  # f32 -> bf16 cast DMA
            # exp in place, per group (2 instrs to pipeline a bit)
            for g in range(G):
                nc.scalar.activation(LT[:, g], LT[:, g], Act.Exp)

            # ---------- blank row -> PB (before the tree destroys LT) ----
            BL = pool.tile([P, G, T], bf16, tag="BL")
            nc.vector.tensor_copy(BL[:], LT[:, :, BLANK, :])

            # ---------- Z = sum over c (f32 accumulation) ---------------
            # strided view [P, G, T, C] so the class dim is innermost
            Z = pool.tile([P, G, T], f32, tag="Z")
            nc.vector.tensor_reduce(
                Z[:], LT[:].transpose([0, 1, 3, 2]), mybir.AxisListType.X,
                AluOp.add)

            # ---------- recipZ * e^kappa ----------
            RZ = pool.tile([P, G, T], f32, tag="RZ")
            nc.vector.reciprocal(RZ[:], Z[:])
            RK = pool.tile([P, G, T], bf16, tag="RK")
            nc.vector.tensor_scalar_mul(RK[:], RZ[:], float(math.exp(KAPPA)))

            # ---------- blank probability series PB ----------
            PB = pool.tile([P, W], bf16, tag="PB")
            nc.vector.tensor_tensor(PB[:, 1:257], BL[:, 0], RK[:, 0], AluOp.mult)
            nc.vector.tensor_tensor(PB[:, 258:514], BL[:, 1], RK[:, 1], AluOp.mult)
            nc.vector.memset(PB[:, 257:258], 0.0)

            # ---------- label series: gather + exp + normalize ----------
            PO = pool.tile([P, L, W], bf16, tag="PO")
            lgt_rows = lgt.ap().rearrange("b c t -> (b c) t")
            GIX = pool.tile([P, NJB * icols], i16, tag="GIX")
            nc.sync.dma_start(GIX[:], gidx.ap())
            for k in range(NJB):
                j0 = k * nj
                Gt = gpool.tile([P, G * nj, T], f32, tag="gbuf")
                nc.gpsimd.dma_gather(
                    Gt[:],
                    lgt_rows,
                    GIX[:, k * icols:(k + 1) * icols],
                    num_idxs=nidx,
                    num_idxs_reg=nidx,
                    elem_size=T,
                )
                # rows r = g*nj + jj  ->  series j0+jj, group g
                nc.scalar.activation(
                    PO[:, j0:j0 + nj, 1:257], Gt[:, 0:nj, :], Act.Exp)
                nc.scalar.activation(
                    PO[:, j0:j0 + nj, 258:514], Gt[:, nj:2 * nj, :], Act.Exp)
                nc.vector.tensor_tensor(
                    PO[:, j0:j0 + nj, 1:257], PO[:, j0:j0 + nj, 1:257],
                    RK[:, 0].unsqueeze(1).broadcast_to([P, nj, T]), AluOp.mult)
                nc.vector.tensor_tensor(
                    PO[:, j0:j0 + nj, 258:514], PO[:, j0:j0 + nj, 258:514],
                    RK[:, 1].unsqueeze(1).broadcast_to([P, nj, T]), AluOp.mult)
            nc.vector.memset(PO[:, :, 257:258], 0.0)

            # ---------- masks / misc tiles ----------
            MC = pool.tile([P, G, L], f32, tag="MC")
            nc.sync.dma_start(MC[:], mc.ap())
            FM = pool.tile([P, S, G], f32, tag="FM")
            nc.sync.dma_start(FM[:], fm.ap())

            IND = pool.tile([P, W], bf16, tag="IND")
            nc.vector.memset(IND[:], 0.0)
            nc.vector.memset(IND[:, 0:1], 1.0)
            nc.vector.memset(IND[:, 257:258], 1.0)

            ROT = [pool.tile([P, W], bf16, tag=f"rot{i}", name=f"rot{i}")
                   for i in range(3)]
            CB = pool.tile([P, W], bf16, tag="CB")
            AM = pool.tile([P, W], bf16, tag="AM")
            FC = pool.tile([P, S, G], bf16, tag="FC")

            # rotating-buffer init slots
            nc.vector.memset(ROT[0][:, 0:1], 1.0)
            nc.vector.memset(ROT[1][:, 0:1], 0.0)
            nc.vector.memset(ROT[2][:, 0:1], 0.0)

            # ---------- the s-chain ----------
            for s in range(S):
                As = ROT[s % 3]
                if s == 0:
                    nc.vector.tensor_tensor_scan(
                        As[:, 1:514], IND[:, 0:513], PB[:, 1:514], 0.0,
                        AluOp.add, AluOp.mult)
                    # A_0 init-indicator fixups for scan_1's data0 reads
                    nc.vector.memset(As[:, 257:258], 1.0)
                elif s == 1:
                    nc.vector.tensor_tensor_scan(
                        As[:, 1:514], ROT[0][:, 0:513], PO[:, 0, 1:514], 0.0,
                        AluOp.add, AluOp.mult)
                    # retire A_0's special slot-0/pad values for its reuse at s=3
                    nc.vector.memset(ROT[0][:, 0:1], 0.0)
                elif s % 2 == 0:
                    nc.vector.tensor_tensor_scan(
                        As[:, 1:514], ROT[(s - 1) % 3][:, 0:513], PB[:, 1:514],
                        0.0, AluOp.add, AluOp.mult)
                else:
                    j = (s - 1) // 2
                    A2 = ROT[(s - 2) % 3]
                    nc.gpsimd.tensor_scalar_mul(
                        AM[:, 0:257], A2[:, 0:257], MC[:, 0, j:j + 1])
                    nc.gpsimd.tensor_scalar_mul(
                        AM[:, 257:514], A2[:, 257:514], MC[:, 1, j:j + 1])
                    nc.vector.tensor_tensor(
                        CB[:, 0:513], ROT[(s - 1) % 3][:, 0:513], AM[:, 0:513],
                        AluOp.add)
                    nc.vector.tensor_tensor_scan(
                        As[:, 1:514], CB[:, 0:513], PO[:, j, 1:514], 0.0,
                        AluOp.add, AluOp.mult)
                # extract final column (t=T-1 of both groups) -> FC[:, s, :]
                base = As[:, 256:514]
                ext_src = bass.AP(base.tensor, base.offset, [list(base.ap[0]), [257, 2]])
                nc.scalar.copy(FC[:, s, :], ext_src)
                if dbg:
                    DT = pool.tile([P, W], f32, tag="DT", name=f"DT{s}")
                    nc.scalar.copy(DT[:], As[:])
                    nc.sync.dma_start(adump.ap()[:, s, :], DT[:])

            # ---------- final: L = sum(FC * FM over the two end states) --
            SEL = pool.tile([P, S, G], f32, tag="SEL")
            nc.vector.tensor_tensor(SEL[:], FC[:], FM[:], AluOp.mult)
            LH = pool.tile([P, G], f32, tag="LH")
            nc.vector.tensor_reduce(
                LH[:], SEL[:].transpose([0, 2, 1]), mybir.AxisListType.X,
                AluOp.add)
            nc.sync.dma_start(lhat.ap(), LH[:])

    nc.compile()
    return nc


def _host_prep(labels):
    """Label-derived index/mask tables (host side; labels are tiny)."""
    lab = np.asarray(labels)
    lsafe = np.where(lab < 0, 0, lab).astype(np.int64)
    label_len = (lab != -1).sum(1)

    per_core = []
    for c in range(NCORES):
        b0 = c * BLOC
        ls = lsafe[b0:b0 + BLOC]          # [256, L]
        ll = label_len[b0:b0 + BLOC]      # [256]

        # gather indices: block k covers label series j = k*nj .. k*nj+nj-1
        nj = L // NJB
        icols = (G * nj * P) // 16
        gidx = np.zeros((P, NJB * icols), np.int16)
        for k in range(NJB):
            flat = np.zeros(G * nj * P, np.int64)
            for i in range(G * nj * P):
                r, p = divmod(i, P)
                g, jj = divmod(r, nj)
                bl = g * P + p
                flat[i] = bl * C + ls[bl, k * nj + jj]
            # wrap: element i lives at partition i%16, col i//16; replicate x8
            cols = flat.reshape(-1, 16).T    # [16, icols]
            for w in range(8):
                gidx[w * 16:(w + 1) * 16, k * icols:(k + 1) * icols] = cols
        # skip-allow scalars m[b, 2j+1]
        mc = np.zeros((P, G, L), np.float32)
        for g in range(G):
            bb = ls[g * P:(g + 1) * P]
            mc[:, g, 1:] = (bb[:, 1:] != bb[:, :-1]).astype(np.float32)
        # final-state selection mask
        fm = np.zeros((P, S, G), np.float32)
        for g in range(G):
            lg = ll[g * P:(g + 1) * P]
            fm[np.arange(P), 2 * lg, g] = 1.0
            fm[np.arange(P), 2 * lg - 1, g] = 1.0
        per_core.append((gidx, mc, fm))
    return per_core


def kernel(labels, logits):
    labels = np.asarray(labels)
    logits = np.asarray(logits, dtype=np.float32)
    assert logits.shape == (B, T, C) and labels.shape == (B, L)

    if "nc" not in _CACHE:
        _CACHE["nc"] = _build_nc()
    nc = _CACHE["nc"]

    tables = _host_prep(labels)
    in_maps = []
    for c in range(NCORES):
        b0 = c * BLOC
        lgt = np.ascontiguousarray(logits[b0:b0 + BLOC].transpose(0, 2, 1))
        gidx, mc, fm = tables[c]
        in_maps.append({"lgt": lgt, "gidx": gidx, "mc": mc, "fm": fm})

    res = run_bass_kernel_spmd(nc, in_maps, core_ids=list(range(NCORES)))
    lh = np.stack([res.results[c]["lhat"] for c in range(NCORES)])  # [8,128,2]
    lh = lh.transpose(0, 2, 1).reshape(-1).astype(np.float64)       # b order
    loss = T * KAPPA - np.log(lh)
    return np.float32(loss.mean())


if __name__ == "__main__":
    import jax

    key = jax.random.key(0)
    k1, k2, k3 = jax.random.split(key, 3)
    logits = np.asarray(jax.random.normal(k1, (B, T, C), dtype=np.float32))
    labels = np.asarray(jax.random.randint(k2, (B, L), 0, C - 1, dtype=np.int32))
    lengths = np.asarray(jax.random.randint(k3, (B,), 16, L + 1, dtype=np.int32))
    labels = np.where(np.arange(L)[None, :] < lengths[:, None], labels, -1)
    print("kernel out:", kernel(labels, logits))


# revision 11
# speedup vs baseline: 1.7657x; 1.7657x over previous
"""CTC loss kernel for Trainium2 (8 NeuronCores, batch-parallel).

Strategy
--------
Data-parallel over batch: each of the 8 cores handles 256 of the 2048
examples (2 "groups" of 128 partition-lanes).

Per core the CTC forward recursion is reformulated as 65 sequential
`tensor_tensor_scan` instructions over the extended-label axis s instead
of 255 tiny per-timestep updates:  for a fixed state s, the recursion
over time  a_t[s] = (a_{t-1}[s] + c_t[s]) * p_t[s]  with
c_t[s] = a_{t-1}[s-1] + m[s]*a_{t-1}[s-2] is exactly the DVE scan
primitive state=(data0+state)*data1 along the free dim, once the series
A_{s-1}, A_{s-2} (computed by the previous scans) are materialized.

Numerics: linear domain with exact softmax normalization folded into the
probabilities (p_hat = exp(logit) * e^kappa / Z_t), where kappa ~= the
mean per-step log-likelihood so alpha stays centered inside fp32 range
(validated: |log alpha| < 62 for this input distribution).
loss_b = T*kappa - log(sum of final alpha states).

Both batch groups ride in one scan instruction via a zero-probability
pad slot that resets the scan state between the groups.

Blank states (even s) all share one probability series (the blank row),
so only the 32 label series are gathered -- via `dma_gather` row-gathers
from a host-side transposed copy of the logits (rows are then
time-contiguous).
"""

import math
import os

import numpy as np

import concourse.bacc as bacc
import concourse.bass as bass
import concourse.mybir as mybir
from concourse import tile
from concourse.bass_utils import run_bass_kernel_spmd

# ---------------- problem constants (hardcoded per the task) -------------
B, T, C, L = 2048, 256, 55, 32
BLANK = C - 1            # 54
S = 2 * L + 1            # 65
NCORES = 8
BLOC = B // NCORES       # 256 examples per core
P = 128                  # partitions
G = 2                    # groups per core
W = 1 + T + 1 + T        # 514 slots: [init | g0 t=0..255 | pad | g1 t=0..255]
KAPPA = 3.73             # drift-centering constant (any value near the mean
                         # per-step log-lik works; exactness not required)
NJB = 8                  # label-series gather blocks
JPB = L // NJB           # 4 label series per block... (see below)

f32 = mybir.dt.float32
bf16 = mybir.dt.bfloat16
i16 = mybir.dt.int16

_CACHE = {}


def _build_nc():
    nc = bacc.Bacc("TRN2", target_bir_lowering=False, debug=False)
    dbg = bool(os.environ.get("CTC_DEBUG_DUMP"))
    nj = L // NJB                    # label series per gather block
    nidx = G * nj * P                # gather indices per block
    icols = nidx // 16               # idx tile columns per block
    lgt = nc.dram_tensor("lgt", [BLOC, C, T], f32, kind="ExternalInput")
    gidx = nc.dram_tensor("gidx", [P, NJB * icols], i16, kind="ExternalInput")
    mc = nc.dram_tensor("mc", [P, G, L], f32, kind="ExternalInput")
    fm = nc.dram_tensor("fm", [P, S, G], f32, kind="ExternalInput")
    lhat = nc.dram_tensor("lhat", [P, G], f32, kind="ExternalOutput")
    adump = (nc.dram_tensor("adump", [P, S, W], f32, kind="ExternalOutput")
             if dbg else None)

    AluOp = mybir.AluOpType
    Act = mybir.ActivationFunctionType

    with tile.TileContext(nc) as tc:
        with (
            tc.tile_pool(name="main", bufs=1) as pool,
            tc.tile_pool(name="gpool", bufs=2) as gpool,
        ):
            # ---------- load + exp(logits^T) ----------
            LT = pool.tile([P, G, C, T], bf16, tag="LT")
            src = lgt.ap().rearrange("(g p) c t -> p g c t", p=P)
            nc.gpsimd.dma_start(LT[:], src)  # f32 -> bf16 cast DMA
            # exp in place, per group (2 instrs to pipeline a bit)
            for g in range(G):
                nc.scalar.activation(LT[:, g], LT[:, g], Act.Exp)

            # ---------- blank row -> PB (before the tree destroys LT) ----
            BL = pool.tile([P, G, T], bf16, tag="BL")
            nc.vector.tensor_copy(BL[:], LT[:, :, BLANK, :])

            # ---------- Z = sum over c: pairwise tree, contiguous reads --
            # lvl1 in place in LT (bf16, 2x mode); f32 accumulation after
            nc.vector.tensor_tensor(
                LT[:, :, 0:27], LT[:, :, 0:27], LT[:, :, 27:54], AluOp.add)
            TZ = pool.tile([P, G, 14, T], f32, tag="TZ")
            nc.vector.tensor_tensor(
                TZ[:, :, 0:13], LT[:, :, 0:13], LT[:, :, 13:26], AluOp.add)
            nc.vector.tensor_tensor(
                TZ[:, :, 13], LT[:, :, 26], LT[:, :, 54], AluOp.add)
            nc.vector.tensor_tensor(
                TZ[:, :, 0:7], TZ[:, :, 0:7], TZ[:, :, 7:14], AluOp.add)
            nc.vector.tensor_tensor(
                TZ[:, :, 0:3], TZ[:, :, 0:3], TZ[:, :, 3:6], AluOp.add)
            nc.vector.tensor_tensor(
                TZ[:, :, 0], TZ[:, :, 0], TZ[:, :, 1], AluOp.add)
            nc.vector.tensor_tensor(
                TZ[:, :, 2], TZ[:, :, 2], TZ[:, :, 6], AluOp.add)
            Z = pool.tile([P, G, T], f32, tag="Z")
            nc.vector.tensor_tensor(
                Z[:], TZ[:, :, 0], TZ[:, :, 2], AluOp.add)

            # ---------- recipZ * e^kappa ----------
            RZ = pool.tile([P, G, T], f32, tag="RZ")
            nc.vector.reciprocal(RZ[:], Z[:])
            RK = pool.tile([P, G, T], bf16, tag="RK")
            nc.vector.tensor_scalar_mul(RK[:], RZ[:], float(math.exp(KAPPA)))

            # ---------- blank probability series PB ----------
            PB = pool.tile([P, W], bf16, tag="PB")
            nc.vector.tensor_tensor(PB[:, 1:257], BL[:, 0], RK[:, 0], AluOp.mult)
            nc.vector.tensor_tensor(PB[:, 258:514], BL[:, 1], RK[:, 1], AluOp.mult)
            nc.vector.memset(PB[:, 257:258], 0.0)

            # ---------- label series: gather + exp + normalize ----------
            PO = pool.tile([P, L, W], bf16, tag="PO")
            lgt_rows = lgt.ap().rearrange("b c t -> (b c) t")
            GIX = pool.tile([P, NJB * icols], i16, tag="GIX")
            nc.sync.dma_start(GIX[:], gidx.ap())
            for k in range(NJB):
                j0 = k * nj
                Gt = gpool.tile([P, G * nj, T], f32, tag="gbuf")
                nc.gpsimd.dma_gather(
                    Gt[:],
                    lgt_rows,
                    GIX[:, k * icols:(k + 1) * icols],
                    num_idxs=nidx,
                    num_idxs_reg=nidx,
                    elem_size=T,
                )
                # rows r = g*nj + jj  ->  series j0+jj, group g
                nc.scalar.activation(
                    PO[:, j0:j0 + nj, 1:257], Gt[:, 0:nj, :], Act.Exp)
                nc.scalar.activation(
                    PO[:, j0:j0 + nj, 258:514], Gt[:, nj:2 * nj, :], Act.Exp)
                nc.vector.tensor_tensor(
                    PO[:, j0:j0 + nj, 1:257], PO[:, j0:j0 + nj, 1:257],
                    RK[:, 0].unsqueeze(1).broadcast_to([P, nj, T]), AluOp.mult)
                nc.vector.tensor_tensor(
                    PO[:, j0:j0 + nj, 258:514], PO[:, j0:j0 + nj, 258:514],
                    RK[:, 1].unsqueeze(1).broadcast_to([P, nj, T]), AluOp.mult)
            nc.vector.memset(PO[:, :, 257:258], 0.0)

            # ---------- masks / misc tiles ----------
            MC = pool.tile([P, G, L], f32, tag="MC")
            nc.sync.dma_start(MC[:], mc.ap())
            FM = pool.tile([P, S, G], f32, tag="FM")
            nc.sync.dma_start(FM[:], fm.ap())

            IND = pool.tile([P, W], bf16, tag="IND")
            nc.vector.memset(IND[:], 0.0)
            nc.vector.memset(IND[:, 0:1], 1.0)
            nc.vector.memset(IND[:, 257:258], 1.0)

            ROT = [pool.tile([P, W], bf16, tag=f"rot{i}", name=f"rot{i}")
                   for i in range(3)]
            CB = pool.tile([P, W], bf16, tag="CB")
            FC = pool.tile([P, S, G], bf16, tag="FC")

            # rotating-buffer init slots
            nc.vector.memset(ROT[0][:, 0:1], 1.0)
            nc.vector.memset(ROT[1][:, 0:1], 0.0)
            nc.vector.memset(ROT[2][:, 0:1], 0.0)

            # ---------- the s-chain ----------
            for s in range(S):
                As = ROT[s % 3]
                if s == 0:
                    nc.vector.tensor_tensor_scan(
                        As[:, 1:514], IND[:, 0:513], PB[:, 1:514], 0.0,
                        AluOp.add, AluOp.mult)
                    # A_0 init-indicator fixups for scan_1's data0 reads
                    nc.vector.memset(As[:, 257:258], 1.0)
                elif s == 1:
                    nc.vector.tensor_tensor_scan(
                        As[:, 1:514], ROT[0][:, 0:513], PO[:, 0, 1:514], 0.0,
                        AluOp.add, AluOp.mult)
                    # retire A_0's special slot-0/pad values for its reuse at s=3
                    nc.vector.memset(ROT[0][:, 0:1], 0.0)
                elif s % 2 == 0:
                    nc.vector.tensor_tensor_scan(
                        As[:, 1:514], ROT[(s - 1) % 3][:, 0:513], PB[:, 1:514],
                        0.0, AluOp.add, AluOp.mult)
                else:
                    j = (s - 1) // 2
                    A2 = ROT[(s - 2) % 3]
                    A1 = ROT[(s - 1) % 3]
                    # CB = A_{s-2} * m + A_{s-1}, fused per group (m is a
                    # per-partition scalar that differs between groups)
                    nc.vector.scalar_tensor_tensor(
                        CB[:, 0:257], A2[:, 0:257], MC[:, 0, j:j + 1],
                        A1[:, 0:257], AluOp.mult, AluOp.add)
                    nc.vector.scalar_tensor_tensor(
                        CB[:, 257:513], A2[:, 257:513], MC[:, 1, j:j + 1],
                        A1[:, 257:513], AluOp.mult, AluOp.add)
                    nc.vector.tensor_tensor_scan(
                        As[:, 1:514], CB[:, 0:513], PO[:, j, 1:514], 0.0,
                        AluOp.add, AluOp.mult)
                # extract final column (t=T-1 of both groups) -> FC[:, s, :]
                base = As[:, 256:514]
                ext_src = bass.AP(base.tensor, base.offset, [list(base.ap[0]), [257, 2]])
                nc.scalar.copy(FC[:, s, :], ext_src)
                if dbg:
                    DT = pool.tile([P, W], f32, tag="DT", name=f"DT{s}")
                    nc.scalar.copy(DT[:], As[:])
                    nc.sync.dma_start(adump.ap()[:, s, :], DT[:])

            # ---------- final: L = sum(FC * FM over the two end states) --
            SEL = pool.tile([P, S, G], f32, tag="SEL")
            nc.vector.tensor_tensor(SEL[:], FC[:], FM[:], AluOp.mult)
            LH = pool.tile([P, G], f32, tag="LH")
            nc.vector.tensor_reduce(
                LH[:], SEL[:].transpose([0, 2, 1]), mybir.AxisListType.X,
                AluOp.add)
            nc.sync.dma_start(lhat.ap(), LH[:])

    nc.compile()
    return nc


def _host_prep(labels):
    """Label-derived index/mask tables (host side; labels are tiny)."""
    lab = np.asarray(labels)
    lsafe = np.where(lab < 0, 0, lab).astype(np.int64)
    label_len = (lab != -1).sum(1)

    per_core = []
    for c in range(NCORES):
        b0 = c * BLOC
        ls = lsafe[b0:b0 + BLOC]          # [256, L]
        ll = label_len[b0:b0 + BLOC]      # [256]

        # gather indices: block k covers label series j = k*nj .. k*nj+nj-1
        nj = L // NJB
        icols = (G * nj * P) // 16
        gidx = np.zeros((P, NJB * icols), np.int16)
        for k in range(NJB):
            flat = np.zeros(G * nj * P, np.int64)
            for i in range(G * nj * P):
                r, p = divmod(i, P)
                g, jj = divmod(r, nj)
                bl = g * P + p
                flat[i] = bl * C + ls[bl, k * nj + jj]
            # wrap: element i lives at partition i%16, col i//16; replicate x8
            cols = flat.reshape(-1, 16).T    # [16, icols]
            for w in range(8):
                gidx[w * 16:(w + 1) * 16, k * icols:(k + 1) * icols] = cols
        # skip-allow scalars m[b, 2j+1]
        mc = np.zeros((P, G, L), np.float32)
        for g in range(G):
            bb = ls[g * P:(g + 1) * P]
            mc[:, g, 1:] = (bb[:, 1:] != bb[:, :-1]).astype(np.float32)
        # final-state selection mask
        fm = np.zeros((P, S, G), np.float32)
        for g in range(G):
            lg = ll[g * P:(g + 1) * P]
            fm[np.arange(P), 2 * lg, g] = 1.0
            fm[np.arange(P), 2 * lg - 1, g] = 1.0
        per_core.append((gidx, mc, fm))
    return per_core


def kernel(labels, logits):
    labels = np.asarray(labels)
    logits = np.asarray(logits, dtype=np.float32)
    assert logits.shape == (B, T, C) and labels.shape == (B, L)

    if "nc" not in _CACHE:
        _CACHE["nc"] = _build_nc()
    nc = _CACHE["nc"]

    tables = _host_prep(labels)
    in_maps = []
    for c in range(NCORES):
        b0 = c * BLOC
        lgt = np.ascontiguousarray(logits[b0:b0 + BLOC].transpose(0, 2, 1))
        gidx, mc, fm = tables[c]
        in_maps.append({"lgt": lgt, "gidx": gidx, "mc": mc, "fm": fm})

    res = run_bass_kernel_spmd(nc, in_maps, core_ids=list(range(NCORES)))
    lh = np.stack([res.results[c]["lhat"] for c in range(NCORES)])  # [8,128,2]
    lh = lh.transpose(0, 2, 1).reshape(-1).astype(np.float64)       # b order
    loss = T * KAPPA - np.log(lh)
    return np.float32(loss.mean())


if __name__ == "__main__":
    import jax

    key = jax.random.key(0)
    k1, k2, k3 = jax.random.split(key, 3)
    logits = np.asarray(jax.random.normal(k1, (B, T, C), dtype=np.float32))
    labels = np.asarray(jax.random.randint(k2, (B, L), 0, C - 1, dtype=np.int32))
    lengths = np.asarray(jax.random.randint(k3, (B,), 16, L + 1, dtype=np.int32))
    labels = np.where(np.arange(L)[None, :] < lengths[:, None], labels, -1)
    print("kernel out:", kernel(labels, logits))


# revision 15
# speedup vs baseline: 1.8125x; 1.0265x over previous
"""CTC loss kernel for Trainium2 (8 NeuronCores, batch-parallel).

Strategy
--------
Data-parallel over batch: each of the 8 cores handles 256 of the 2048
examples (2 "groups" of 128 partition-lanes).

Per core the CTC forward recursion is reformulated as 65 sequential
`tensor_tensor_scan` instructions over the extended-label axis s instead
of 255 tiny per-timestep updates:  for a fixed state s, the recursion
over time  a_t[s] = (a_{t-1}[s] + c_t[s]) * p_t[s]  with
c_t[s] = a_{t-1}[s-1] + m[s]*a_{t-1}[s-2] is exactly the DVE scan
primitive state=(data0+state)*data1 along the free dim, once the series
A_{s-1}, A_{s-2} (computed by the previous scans) are materialized.

Numerics: linear domain with exact softmax normalization folded into the
probabilities (p_hat = exp(logit) * e^kappa / Z_t), where kappa ~= the
mean per-step log-likelihood so alpha stays centered inside fp32 range
(validated: |log alpha| < 62 for this input distribution).
loss_b = T*kappa - log(sum of final alpha states).

Both batch groups ride in one scan instruction via a zero-probability
pad slot that resets the scan state between the groups.

Blank states (even s) all share one probability series (the blank row),
so only the 32 label series are gathered -- via `dma_gather` row-gathers
from a host-side transposed copy of the logits (rows are then
time-contiguous).
"""

import math
import os

import numpy as np

import concourse.bacc as bacc
import concourse.bass as bass
import concourse.mybir as mybir
from concourse import tile
from concourse.bass_utils import run_bass_kernel_spmd

# ---------------- problem constants (hardcoded per the task) -------------
B, T, C, L = 2048, 256, 55, 32
BLANK = C - 1            # 54
S = 2 * L + 1            # 65
NCORES = 8
BLOC = B // NCORES       # 256 examples per core
P = 128                  # partitions
G = 2                    # groups per core
W = 1 + T + 1 + T        # 514 slots: [init | g0 t=0..255 | pad | g1 t=0..255]
KAPPA = 3.73             # drift-centering constant (any value near the mean
                         # per-step log-lik works; exactness not required)
NJB = 8                  # label-series gather blocks
JPB = L // NJB           # 4 label series per block... (see below)

f32 = mybir.dt.float32
bf16 = mybir.dt.bfloat16
i16 = mybir.dt.int16

_CACHE = {}


def _build_nc():
    nc = bacc.Bacc("TRN2", target_bir_lowering=False, debug=False,
                   dynamic_dma_scratch_size=32768)
    dbg = bool(os.environ.get("CTC_DEBUG_DUMP"))
    nj = L // NJB                    # label series per gather block
    nidx = G * nj * P                # gather indices per block
    icols = nidx // 16               # idx tile columns per block
    lgt = nc.dram_tensor("lgt", [BLOC, C, T], f32, kind="ExternalInput")
    gidx = nc.dram_tensor("gidx", [P, NJB * icols], i16, kind="ExternalInput")
    mc = nc.dram_tensor("mc", [P, G, L], f32, kind="ExternalInput")
    fm = nc.dram_tensor("fm", [P, S, G], f32, kind="ExternalInput")
    lhat = nc.dram_tensor("lhat", [P, G], f32, kind="ExternalOutput")
    adump = (nc.dram_tensor("adump", [P, S, W], f32, kind="ExternalOutput")
             if dbg else None)

    AluOp = mybir.AluOpType
    Act = mybir.ActivationFunctionType

    with tile.TileContext(nc) as tc:
        with (
            tc.tile_pool(name="main", bufs=1) as pool,
            tc.tile_pool(name="gpool", bufs=2) as gpool,
        ):
            # ---------- load + exp + Z + recip + PB, pipelined in T-chunks
            LT = pool.tile([P, G, C, T], bf16, tag="LT")
            TZ = pool.tile([P, G, 14, 128], f32, tag="TZ")
            Z = pool.tile([P, G, T], f32, tag="Z")
            RZ = pool.tile([P, G, T], f32, tag="RZ")
            RK = pool.tile([P, G, T], bf16, tag="RK")
            PB = pool.tile([P, W], bf16, tag="PB")
            src = lgt.ap().rearrange("(g p) c t -> p g c t", p=P)
            TC = 128  # >=128 keeps DMA runs at 512B
            for ch in range(T // TC):
                t0, t1 = ch * TC, (ch + 1) * TC
                cs = slice(t0, t1)
                for g in range(G):
                    nc.gpsimd.dma_start(LT[:, g, :, cs], src[:, g, :, cs])
                    nc.scalar.activation(LT[:, g, :, cs], LT[:, g, :, cs], Act.Exp)
                # Z tree: lvl1 in place (bf16 2x), then f32 accumulation
                nc.vector.tensor_tensor(
                    LT[:, :, 0:27, cs], LT[:, :, 0:27, cs], LT[:, :, 27:54, cs],
                    AluOp.add)
                nc.vector.tensor_tensor(
                    TZ[:, :, 0:13], LT[:, :, 0:13, cs], LT[:, :, 13:26, cs],
                    AluOp.add)
                nc.vector.tensor_tensor(
                    TZ[:, :, 13], LT[:, :, 26, cs], LT[:, :, 54, cs],
                    AluOp.add)
                nc.vector.tensor_tensor(
                    TZ[:, :, 0:7], TZ[:, :, 0:7], TZ[:, :, 7:14], AluOp.add)
                nc.vector.tensor_tensor(
                    TZ[:, :, 0:3], TZ[:, :, 0:3], TZ[:, :, 3:6], AluOp.add)
                nc.vector.tensor_tensor(
                    TZ[:, :, 0], TZ[:, :, 0], TZ[:, :, 1], AluOp.add)
                nc.vector.tensor_tensor(
                    TZ[:, :, 2], TZ[:, :, 2], TZ[:, :, 6], AluOp.add)
                nc.vector.tensor_tensor(
                    Z[:, :, cs], TZ[:, :, 0], TZ[:, :, 2], AluOp.add)
                nc.vector.reciprocal(RZ[:, :, cs], Z[:, :, cs])
                nc.vector.tensor_scalar_mul(
                    RK[:, :, cs], RZ[:, :, cs], float(math.exp(KAPPA)))
                # blank prob series slots for this chunk (row 54 survives lvl1)
                nc.vector.tensor_tensor(
                    PB[:, 1 + t0:1 + t1], LT[:, 0, 54, cs], RK[:, 0, cs],
                    AluOp.mult)
                nc.vector.tensor_tensor(
                    PB[:, 258 + t0:258 + t1], LT[:, 1, 54, cs], RK[:, 1, cs],
                    AluOp.mult)
            nc.vector.memset(PB[:, 257:258], 0.0)

            # ---------- label series: gather + exp + normalize ----------
            PO = pool.tile([P, L, W], bf16, tag="PO")
            lgt_rows = lgt.ap().rearrange("b c t -> (b c) t")
            GIX = pool.tile([P, NJB * icols], i16, tag="GIX")
            nc.sync.dma_start(GIX[:], gidx.ap())
            for k in range(NJB):
                j0 = k * nj
                Gt = gpool.tile([P, G * nj, T], f32, tag="gbuf")
                nc.gpsimd.dma_gather(
                    Gt[:],
                    lgt_rows,
                    GIX[:, k * icols:(k + 1) * icols],
                    num_idxs=nidx,
                    num_idxs_reg=nidx,
                    elem_size=T,
                )
                # rows r = g*nj + jj  ->  series j0+jj, group g
                nc.scalar.activation(
                    PO[:, j0:j0 + nj, 1:257], Gt[:, 0:nj, :], Act.Exp)
                nc.scalar.activation(
                    PO[:, j0:j0 + nj, 258:514], Gt[:, nj:2 * nj, :], Act.Exp)
                nc.vector.tensor_tensor(
                    PO[:, j0:j0 + nj, 1:257], PO[:, j0:j0 + nj, 1:257],
                    RK[:, 0].unsqueeze(1).broadcast_to([P, nj, T]), AluOp.mult)
                nc.vector.tensor_tensor(
                    PO[:, j0:j0 + nj, 258:514], PO[:, j0:j0 + nj, 258:514],
                    RK[:, 1].unsqueeze(1).broadcast_to([P, nj, T]), AluOp.mult)
            nc.vector.memset(PO[:, :, 257:258], 0.0)

            # ---------- masks / misc tiles ----------
            MC = pool.tile([P, G, L], f32, tag="MC")
            nc.sync.dma_start(MC[:], mc.ap())
            FM = pool.tile([P, S, G], f32, tag="FM")
            nc.sync.dma_start(FM[:], fm.ap())

            IND = pool.tile([P, W], bf16, tag="IND")
            nc.vector.memset(IND[:], 0.0)
            nc.vector.memset(IND[:, 0:1], 1.0)
            nc.vector.memset(IND[:, 257:258], 1.0)

            ROT = [pool.tile([P, W], bf16, tag=f"rot{i}", name=f"rot{i}")
                   for i in range(3)]
            CB = pool.tile([P, W], bf16, tag="CB")
            FC = pool.tile([P, S, G], bf16, tag="FC")

            # rotating-buffer init slots
            nc.vector.memset(ROT[0][:, 0:1], 1.0)
            nc.vector.memset(ROT[1][:, 0:1], 0.0)
            nc.vector.memset(ROT[2][:, 0:1], 0.0)

            # ---------- the s-chain ----------
            for s in range(S):
                As = ROT[s % 3]
                if s == 0:
                    nc.vector.tensor_tensor_scan(
                        As[:, 1:514], IND[:, 0:513], PB[:, 1:514], 0.0,
                        AluOp.add, AluOp.mult)
                    # A_0 init-indicator fixups for scan_1's data0 reads
                    nc.vector.memset(As[:, 257:258], 1.0)
                elif s == 1:
                    nc.vector.tensor_tensor_scan(
                        As[:, 1:514], ROT[0][:, 0:513], PO[:, 0, 1:514], 0.0,
                        AluOp.add, AluOp.mult)
                    # retire A_0's special slot-0/pad values for its reuse at s=3
                    nc.vector.memset(ROT[0][:, 0:1], 0.0)
                elif s % 2 == 0:
                    nc.vector.tensor_tensor_scan(
                        As[:, 1:514], ROT[(s - 1) % 3][:, 0:513], PB[:, 1:514],
                        0.0, AluOp.add, AluOp.mult)
                else:
                    j = (s - 1) // 2
                    A2 = ROT[(s - 2) % 3]
                    A1 = ROT[(s - 1) % 3]
                    # CB = A_{s-2} * m + A_{s-1}, fused per group (m is a
                    # per-partition scalar that differs between groups)
                    nc.vector.scalar_tensor_tensor(
                        CB[:, 0:257], A2[:, 0:257], MC[:, 0, j:j + 1],
                        A1[:, 0:257], AluOp.mult, AluOp.add)
                    nc.vector.scalar_tensor_tensor(
                        CB[:, 257:513], A2[:, 257:513], MC[:, 1, j:j + 1],
                        A1[:, 257:513], AluOp.mult, AluOp.add)
                    nc.vector.tensor_tensor_scan(
                        As[:, 1:514], CB[:, 0:513], PO[:, j, 1:514], 0.0,
                        AluOp.add, AluOp.mult)
                # extract final column (t=T-1 of both groups) -> FC[:, s, :]
                base = As[:, 256:514]
                ext_src = bass.AP(base.tensor, base.offset, [list(base.ap[0]), [257, 2]])
                nc.scalar.copy(FC[:, s, :], ext_src)
                if dbg:
                    DT = pool.tile([P, W], f32, tag="DT", name=f"DT{s}")
                    nc.scalar.copy(DT[:], As[:])
                    nc.sync.dma_start(adump.ap()[:, s, :], DT[:])

            # ---------- final: L = sum(FC * FM over the two end states) --
            SEL = pool.tile([P, S, G], f32, tag="SEL")
            nc.vector.tensor_tensor(SEL[:], FC[:], FM[:], AluOp.mult)
            LH = pool.tile([P, G], f32, tag="LH")
            nc.vector.tensor_reduce(
                LH[:], SEL[:].transpose([0, 2, 1]), mybir.AxisListType.X,
                AluOp.add)
            nc.sync.dma_start(lhat.ap(), LH[:])

    nc.compile()
    return nc


def _host_prep(labels):
    """Label-derived index/mask tables (host side; labels are tiny)."""
    lab = np.asarray(labels)
    lsafe = np.where(lab < 0, 0, lab).astype(np.int64)
    label_len = (lab != -1).sum(1)

    per_core = []
    for c in range(NCORES):
        b0 = c * BLOC
        ls = lsafe[b0:b0 + BLOC]          # [256, L]
        ll = label_len[b0:b0 + BLOC]      # [256]

        # gather indices: block k covers label series j = k*nj .. k*nj+nj-1
        nj = L // NJB
        icols = (G * nj * P) // 16
        gidx = np.zeros((P, NJB * icols), np.int16)
        for k in range(NJB):
            flat = np.zeros(G * nj * P, np.int64)
            for i in range(G * nj * P):
                r, p = divmod(i, P)
                g, jj = divmod(r, nj)
                bl = g * P + p
                flat[i] = bl * C + ls[bl, k * nj + jj]
            # wrap: element i lives at partition i%16, col i//16; replicate x8
            cols = flat.reshape(-1, 16).T    # [16, icols]
            for w in range(8):
                gidx[w * 16:(w + 1) * 16, k * icols:(k + 1) * icols] = cols
        # skip-allow scalars m[b, 2j+1]
        mc = np.zeros((P, G, L), np.float32)
        for g in range(G):
            bb = ls[g * P:(g + 1) * P]
            mc[:, g, 1:] = (bb[:, 1:] != bb[:, :-1]).astype(np.float32)
        # final-state selection mask
        fm = np.zeros((P, S, G), np.float32)
        for g in range(G):
            lg = ll[g * P:(g + 1) * P]
            fm[np.arange(P), 2 * lg, g] = 1.0
            fm[np.arange(P), 2 * lg - 1, g] = 1.0
        per_core.append((gidx, mc, fm))
    return per_core


def kernel(labels, logits):
    labels = np.asarray(labels)
    logits = np.asarray(logits, dtype=np.float32)
    assert logits.shape == (B, T, C) and labels.shape == (B, L)

    if "nc" not in _CACHE:
        _CACHE["nc"] = _build_nc()
    nc = _CACHE["nc"]

    tables = _host_prep(labels)
    in_maps = []
    for c in range(NCORES):
        b0 = c * BLOC
        lgt = np.ascontiguousarray(logits[b0:b0 + BLOC].transpose(0, 2, 1))
        gidx, mc, fm = tables[c]
        in_maps.append({"lgt": lgt, "gidx": gidx, "mc": mc, "fm": fm})

    res = run_bass_kernel_spmd(nc, in_maps, core_ids=list(range(NCORES)))
    lh = np.stack([res.results[c]["lhat"] for c in range(NCORES)])  # [8,128,2]
    lh = lh.transpose(0, 2, 1).reshape(-1).astype(np.float64)       # b order
    loss = T * KAPPA - np.log(lh)
    return np.float32(loss.mean())


if __name__ == "__main__":
    import jax

    key = jax.random.key(0)
    k1, k2, k3 = jax.random.split(key, 3)
    logits = np.asarray(jax.random.normal(k1, (B, T, C), dtype=np.float32))
    labels = np.asarray(jax.random.randint(k2, (B, L), 0, C - 1, dtype=np.int32))
    lengths = np.asarray(jax.random.randint(k3, (B,), 16, L + 1, dtype=np.int32))
    labels = np.where(np.arange(L)[None, :] < lengths[:, None], labels, -1)
    print("kernel out:", kernel(labels, logits))
